# revision 11
# baseline (speedup 1.0000x reference)
"""BiMamba block Trainium2 kernel (v3: 3-chunk pipeline, DMA bcast,
gpsimd ch-multiply).

Sharding: 8 cores = (direction f/b) x (batch 0/1) x (d_inner half 0/1),
fully independent (no collectives).  Host flips the sequence for the
backward cores, relabels u channel tiles so this core's half occupies
m=0..7, and sums the 8 partial outputs + residual.

Per core the sequence is processed in 3 time chunks [256, 384, 384] so
the exposed PE-heavy front of chunk 0 is small and the front of chunk
c+1 overlaps the DVE-bound scan of chunk c.  The front emission for
chunk c+1 is sliced across the slab loop of chunk c so its scalar ops
do not head-of-line block the asl exps feeding the scans.

The selective scan runs per (chunk, state-group g, m-tile): d on
partitions, 8 state segments x LC t in the free dim, one
tensor_tensor_scan per slab; chunk boundaries are stitched by folding
a*h_prev into the first column of b.  B/C rows are broadcast to 128
partitions via a DRAM bounce + replicated-read DMAs (no PE/scalar).
The C*h multiply runs on gpsimd; the n-segment reduction
sum_j C_j*h_j + D*u runs on the PE as accumulating identity/diag
matmuls into PSUM.
"""

import sys

sys.path.insert(0, "/opt/trn_rl_repo")

import numpy as np

import concourse.bass as bass
import concourse.mybir as mybir
from concourse import bacc
from concourse.tile import TileContext
from concourse.bass_utils import run_bass_kernel_spmd

FP32 = mybir.dt.float32
BF16 = mybir.dt.bfloat16
AX = mybir.AxisListType
OP = mybir.AluOpType
AF = mybir.ActivationFunctionType

P = 128
L = 1024          # sequence length
DM = 1024         # d_model
DI = 2048         # d_inner
DH = 1024         # d_inner half per core
DSTATE = 16
DTRANK = 64
DCONV = 4
NKM = DM // P     # 8 d_model tiles
NMU = DI // P     # 16 u M-tiles
NMH = DH // P     # 8 half M-tiles
GSEG = 8          # states per scan slab
CHUNKS = [(0, 256), (256, 384), (640, 384)]
NCH = len(CHUNKS)
LCMAX = max(lc for _, lc in CHUNKS)
NTCMAX = (LCMAX + P - 1) // P
SLABMAX = GSEG * LCMAX


def build_program(finalize=True):
    nc = bacc.Bacc("TRN2", target_bir_lowering=False, debug=False)

    # ---- DRAM I/O (per-core shards; same names on every core) ----
    xin = nc.dram_tensor("xin", (L, DM), FP32, kind="ExternalInput")
    wuT = nc.dram_tensor("wuT", (NMU, P, NKM * P), BF16, kind="ExternalInput")
    wzT = nc.dram_tensor("wzT", (NMH, P, NKM * P), BF16, kind="ExternalInput")
    bu = nc.dram_tensor("bu", (P, NMU), FP32, kind="ExternalInput")
    bz = nc.dram_tensor("bz", (P, NMH), FP32, kind="ExternalInput")
    convd = nc.dram_tensor("convd", (NMU, P, DCONV * P), BF16, kind="ExternalInput")
    convb = nc.dram_tensor("convb", (P, NMU), FP32, kind="ExternalInput")
    wxpT = nc.dram_tensor("wxpT", (P, NMU * 96), BF16, kind="ExternalInput")
    wdtT = nc.dram_tensor("wdtT", (DTRANK, DH), BF16, kind="ExternalInput")
    dtb = nc.dram_tensor("dtb", (P, NMH), FP32, kind="ExternalInput")
    Amat = nc.dram_tensor("Amat", (P, P), FP32, kind="ExternalInput")
    Ddiag = nc.dram_tensor("Ddiag", (NMH, P, P), BF16, kind="ExternalInput")
    woT = nc.dram_tensor("woT", (NKM, P, NMH * P), BF16, kind="ExternalInput")
    ident = nc.dram_tensor("ident", (P, P), BF16, kind="ExternalInput")
    outp = nc.dram_tensor("outp", (DM, L), FP32, kind="ExternalOutput")

    with TileContext(nc) as tc:
        const = tc.alloc_tile_pool(name="const", bufs=1)
        main = tc.alloc_tile_pool(name="main", bufs=1)
        wpool = tc.alloc_tile_pool(name="wpool", bufs=2)
        ppool = tc.alloc_tile_pool(name="ppool", bufs=3, space="PSUM")
        ypool = tc.alloc_tile_pool(name="ypool", bufs=2, space="PSUM")
        trpool = tc.alloc_tile_pool(name="trpool", bufs=2, space="PSUM")
        front = tc.alloc_tile_pool(name="front", bufs=2)
        scanp = tc.alloc_tile_pool(name="scanp", bufs=1)
        dramp = tc.alloc_tile_pool(name="dramp", bufs=2, space="DRAM")

        bu_t = const.tile((P, NMU), FP32, name="bu_t")
        bz_t = const.tile((P, NMH), FP32, name="bz_t")
        convb_t = const.tile((P, NMU), FP32, name="convb_t")
        dtb_t = const.tile((P, NMH), FP32, name="dtb_t")
        A_t = const.tile((P, P), FP32, name="A_t")
        id_t = const.tile((P, P), BF16, name="id_t")
        dd_t = const.tile((P, NMH * P), BF16, name="dd_t")
        wdt = const.tile((DTRANK, DH), BF16, name="wdt")
        wxp_t = const.tile((P, NMU * 96), BF16, name="wxp_t")

        def emit_consts():
            for dst, srct in ((bu_t, bu), (bz_t, bz), (convb_t, convb),
                              (dtb_t, dtb), (A_t, Amat), (id_t, ident)):
                nc.sync.dma_start(out=dst[:], in_=srct[:])
            for m in range(NMH):
                nc.sync.dma_start(out=dd_t[:, m * P:(m + 1) * P], in_=Ddiag[m])
            nc.sync.dma_start(out=wdt[:], in_=wdtT[:])
            nc.sync.dma_start(out=wxp_t[:], in_=wxpT[:])

        # persistent across chunks
        u_pre = [main.tile((P, LCMAX + 4), BF16, name=f"up{m}", tag=f"up{m}")
                 for m in range(NMU)]
        hlast = [main.tile((P, DSTATE), BF16, name=f"hl{m}", tag=f"hl{m}")
                 for m in range(NMH)]

        # ---------------- front phase builders ----------------
        def emit_xdma(ci):
            c0, LC = CHUNKS[ci]
            xts = []
            for tt in range(LC // P):
                t0 = c0 + tt * P
                xt = front.tile((P, DM), FP32, name="xt", tag="xt", bufs=3)
                for kk in range(NKM):
                    nc.sync.dma_start(out=xt[:, kk * P:(kk + 1) * P],
                                      in_=xin[t0:t0 + P, kk * P:(kk + 1) * P])
                xts.append(xt)
            return xts

        def emit_ln(ci, xts):
            c0, LC = CHUNKS[ci]
            ntc = LC // P
            xnT = [front.tile((P, LCMAX), BF16, name=f"xnT{k}", tag=f"xnT{k}",
                              bufs=2) for k in range(NKM)]
            mus = front.tile((P, NTCMAX), FP32, name="mus", tag="mus", bufs=2)
            vars_ = front.tile((P, NTCMAX), FP32, name="vars", tag="vars",
                               bufs=2)
            for tt in range(ntc):
                xt = xts[tt]
                bns = front.tile((P, 12), FP32, name="bns", tag="stats", bufs=8)
                nc.vector.bn_stats(bns[:, 0:6], xt[:, 0:DM // 2])
                nc.vector.bn_stats(bns[:, 6:12], xt[:, DM // 2:DM])
                mv = front.tile((P, 2), FP32, name="mv", tag="stats", bufs=8)
                nc.vector.bn_aggr(mv[:], bns[:])
                nc.vector.tensor_copy(mus[:, tt:tt + 1], mv[:, 0:1])
                nc.vector.tensor_scalar_add(vars_[:, tt:tt + 1], mv[:, 1:2],
                                            1e-5)
            sds = front.tile((P, NTCMAX), FP32, name="sds", tag="stats", bufs=8)
            nc.scalar.activation(sds[:, :ntc], vars_[:, :ntc], AF.Sqrt)
            rs = front.tile((P, NTCMAX), FP32, name="rs", tag="rs", bufs=2)
            nc.vector.reciprocal(rs[:, :ntc], sds[:, :ntc])
            for tt in range(ntc):
                xn = front.tile((P, DM), BF16, name="xn", tag="xn", bufs=2)
                nc.vector.tensor_scalar(xn[:], xts[tt][:], mus[:, tt:tt + 1],
                                        rs[:, tt:tt + 1], OP.subtract, OP.mult)
                for kk in range(NKM):
                    tr = trpool.tile((P, P), BF16, name="tr", tag="tr")
                    nc.tensor.transpose(tr[:], xn[:, kk * P:(kk + 1) * P],
                                        id_t[:])
                    if ci == 0:
                        nc.vector.tensor_copy(
                            xnT[kk][:, tt * P:(tt + 1) * P], tr[:])
                    else:
                        nc.scalar.copy(xnT[kk][:, tt * P:(tt + 1) * P], tr[:])
            return xnT

        def emit_uprojA(ci, m, xnT):
            """in_proj u tile m -> u_pre (pre-conv)."""
            c0, LC = CHUNKS[ci]
            wt = wpool.tile((P, NKM * P), BF16, name="wt", tag="wt", bufs=2)
            nc.sync.dma_start(out=wt[:], in_=wuT[m])
            ps = ppool.tile((P, LCMAX), FP32, name="ps", tag="mm")
            for k in range(NKM):
                nc.tensor.matmul(ps[:, :LC], wt[:, k * P:(k + 1) * P],
                                 xnT[k][:, :LC],
                                 start=(k == 0), stop=(k == NKM - 1))
            if ci == 0:
                nc.vector.memset(u_pre[m][:, 0:4], 0.0)
            else:
                lcp = CHUNKS[ci - 1][1]
                nc.scalar.copy(u_pre[m][:, 1:4], u_pre[m][:, lcp + 1:lcp + 4])
            nc.scalar.activation(u_pre[m][:, 4:LC + 4], ps[:, :LC],
                                 AF.Identity, bias=bu_t[:, m:m + 1])

        def emit_conv(ci, m, S):
            """conv + silu of u tile m."""
            c0, LC = CHUNKS[ci]
            pc = ppool.tile((P, LCMAX), FP32, name="pc", tag="mm")
            cw = wpool.tile((P, DCONV * P), BF16, name="cw", tag="cw", bufs=2)
            nc.sync.dma_start(out=cw[:], in_=convd[m])
            for k in range(DCONV):
                nc.tensor.matmul(pc[:, :LC], cw[:, k * P:(k + 1) * P],
                                 u_pre[m][:, k + 1:k + 1 + LC],
                                 start=(k == 0), stop=(k == DCONV - 1))
            if m < NMH:
                us = front.tile((P, LCMAX), BF16, name=f"usl{m}",
                                tag=f"usl{m}", bufs=2)
                S["u_silu"][m] = us
            else:
                us = front.tile((P, LCMAX), BF16, name="uslB", tag="uslB",
                                bufs=2)
            S.setdefault("u_all", [None] * NMU)[m] = us
            nc.scalar.activation(us[:, :LC], pc[:, :LC], AF.Silu,
                                 bias=convb_t[:, m:m + 1])

        def emit_xproj(ci, m, pxp, S):
            c0, LC = CHUNKS[ci]
            nc.tensor.matmul(pxp[:, :LC], wxp_t[:, m * 96:(m + 1) * 96],
                             S["u_all"][m][:, :LC],
                             start=(m == 0), stop=(m == NMU - 1))

        def emit_dt(ci, pxp, S):
            c0, LC = CHUNKS[ci]
            dbc = front.tile((96, LCMAX), BF16, name="dbc", tag="dbc", bufs=2)
            S["dbc"] = dbc
            nc.scalar.copy(dbc[:, :LC], pxp[:, :LC])
            for m in range(NMH):
                psd = ppool.tile((P, LCMAX), FP32, name="psd", tag="mm")
                nc.tensor.matmul(psd[:, :LC], wdt[:, m * P:(m + 1) * P],
                                 dbc[0:DTRANK, :LC], start=True, stop=True)
                dts = front.tile((P, LCMAX), BF16, name=f"dts{m}",
                                 tag=f"dts{m}", bufs=2)
                S["dt_sb"][m] = dts
                nc.scalar.activation(dts[:, :LC], psd[:, :LC], AF.Exp,
                                     bias=dtb_t[:, m:m + 1])
            for m in range(NMH):
                dts = S["dt_sb"][m]
                nc.scalar.activation(dts[:, :LC], dts[:, :LC], AF.Ln, bias=1.0)

        def emit_z(ci, xnT, S):
            c0, LC = CHUNKS[ci]
            for m in range(NMH):
                wt = wpool.tile((P, NKM * P), BF16, name="wtz", tag="wt",
                                bufs=2)
                nc.sync.dma_start(out=wt[:], in_=wzT[m])
                ps = ppool.tile((P, LCMAX), FP32, name="psz", tag="mm")
                for k in range(NKM):
                    nc.tensor.matmul(ps[:, :LC], wt[:, k * P:(k + 1) * P],
                                     xnT[k][:, :LC],
                                     start=(k == 0), stop=(k == NKM - 1))
                zs = front.tile((P, LCMAX), BF16, name=f"zsl{m}",
                                tag=f"zsl{m}", bufs=2)
                S["z_silu"][m] = zs
                nc.scalar.activation(zs[:, :LC], ps[:, :LC], AF.Silu,
                                     bias=bz_t[:, m:m + 1])

        # ---------------- scan phase builders ----------------
        def emit_bcast(ci, S):
            """Broadcast the 32 B/C rows of dbc to 128 partitions via a DRAM
            bounce + replicated-read DMAs (no PE / scalar involvement)."""
            c0, LC = CHUNKS[ci]
            dbc = S["dbc"]
            bcd = dramp.tile((32, LCMAX), BF16, name="bcd", tag="bcd", bufs=2)
            nc.sync.dma_start(out=bcd[:, :LC], in_=dbc[DTRANK:DTRANK + 32, :LC])
            slabs = []
            for g in range(2):
                Bsl = scanp.tile((P, SLABMAX), BF16, name="Bsl",
                                 tag=f"B{g}{ci % 2}", bufs=1)
                Csl = scanp.tile((P, SLABMAX), BF16, name="Csl",
                                 tag=f"C{g}{ci % 2}", bufs=1)
                for j in range(GSEG):
                    n = g * GSEG + j
                    nc.sync.dma_start(
                        out=Bsl[:, j * LC:(j + 1) * LC],
                        in_=bcd[n:n + 1, :LC].partition_broadcast(P))
                    nc.sync.dma_start(
                        out=Csl[:, j * LC:(j + 1) * LC],
                        in_=bcd[16 + n:16 + n + 1, :LC].partition_broadcast(P))
                slabs.append((Bsl, Csl))
            return slabs

        def emit_slab(ci, m, bc, S):
            """Both state groups of m-tile m for chunk ci + PE reduce + gate."""
            c0, LC = CHUNKS[ci]
            SLABF = GSEG * LC
            dtu = scanp.tile((P, LCMAX), BF16, name="dtu", tag="dtu", bufs=2)
            nc.vector.tensor_mul(dtu[:, :LC], S["dt_sb"][m][:, :LC],
                                 S["u_silu"][m][:, :LC])
            yps = ypool.tile((P, LCMAX), FP32, name="yps", tag="yps")
            for g in range(2):
                Bsl, Csl = bc[g]
                asl = scanp.tile((P, SLABMAX), BF16, name="asl", tag="asl",
                                 bufs=2)
                for j in range(GSEG):
                    n = g * GSEG + j
                    nc.scalar.activation(
                        asl[:, j * LC:(j + 1) * LC], S["dt_sb"][m][:, :LC],
                        AF.Exp,
                        scale=A_t[:, m * DSTATE + n:m * DSTATE + n + 1])
                bsl = scanp.tile((P, SLABMAX), BF16, name="bsl", tag="bsl",
                                 bufs=2)
                nc.vector.tensor_tensor(
                    bsl[:, :SLABF].rearrange("p (j t) -> p j t", j=GSEG),
                    dtu[:, :LC].unsqueeze(1).to_broadcast((P, GSEG, LC)),
                    Bsl[:, :SLABF].rearrange("p (j t) -> p j t", j=GSEG),
                    OP.mult)
                if ci == 0:
                    nc.vector.memset(asl[:, 0:SLABF:LC], 0.0)
                else:
                    tmp8 = scanp.tile((P, GSEG), BF16, name="tmp8", tag="tmp8",
                                      bufs=2)
                    nc.vector.tensor_tensor(tmp8[:], asl[:, 0:SLABF:LC],
                                            hlast[m][:, g * GSEG:(g + 1) * GSEG],
                                            OP.mult)
                    nc.vector.tensor_tensor(bsl[:, 0:SLABF:LC],
                                            bsl[:, 0:SLABF:LC], tmp8[:],
                                            OP.add)
                    nc.vector.memset(asl[:, 0:SLABF:LC], 0.0)
                hsl = scanp.tile((P, SLABMAX), BF16, name="hsl", tag="hsl",
                                 bufs=2)
                nc.vector.tensor_tensor_scan(hsl[:, :SLABF], asl[:, :SLABF],
                                             bsl[:, :SLABF], 0.0,
                                             OP.mult, OP.add)
                if ci < NCH - 1:
                    nc.vector.tensor_copy(
                        hlast[m][:, g * GSEG:(g + 1) * GSEG],
                        hsl[:, LC - 1:SLABF:LC])
                ch = scanp.tile((P, SLABMAX), BF16, name="ch", tag="chs",
                                bufs=2)
                nc.vector.tensor_mul(ch[:, :SLABF], hsl[:, :SLABF],
                                     Csl[:, :SLABF])
                for j in range(GSEG):
                    nc.tensor.matmul(yps[:, :LC], id_t[:],
                                     ch[:, j * LC:(j + 1) * LC],
                                     start=(g == 0 and j == 0), stop=False)
            nc.tensor.matmul(yps[:, :LC], dd_t[:, m * P:(m + 1) * P],
                             S["u_silu"][m][:, :LC], start=False, stop=True)
            return yps

        def emit_gate(ci, m, S, yps):
            c0, LC = CHUNKS[ci]
            yg = front.tile((P, LCMAX), BF16, name=f"yg{m}", tag=f"yg{m}",
                            bufs=1)
            S["ygate"][m] = yg
            nc.vector.tensor_tensor(yg[:, :LC], yps[:, :LC],
                                    S["z_silu"][m][:, :LC], OP.mult)

        def emit_outproj_kk(ci, kk, S):
            c0, LC = CHUNKS[ci]
            wt = wpool.tile((P, NMH * P), BF16, name="wto", tag="wto",
                            bufs=2)
            nc.sync.dma_start(out=wt[:], in_=woT[kk])
            po = ppool.tile((P, LCMAX), FP32, name="po", tag="mm")
            for m in range(NMH):
                nc.tensor.matmul(po[:, :LC], wt[:, m * P:(m + 1) * P],
                                 S["ygate"][m][:, :LC],
                                 start=(m == 0), stop=(m == NMH - 1))
            osb = front.tile((P, LCMAX), FP32, name="osb", tag="osb",
                             bufs=2)
            nc.scalar.copy(osb[:, :LC], po[:, :LC])
            nc.sync.dma_start(out=outp[kk * P:(kk + 1) * P, c0:c0 + LC],
                              in_=osb[:, :LC])

        def front_slices(ci, S):
            """Emission slices for chunk ci's front, to be spread across the
            previous chunk's slab loop (keeps scalar ahead of the DVE)."""
            pxp = ppool.tile((96, LCMAX), FP32, name="pxp", tag="pxp", bufs=1)
            st = {}

            def s_ln():
                st["xnT"] = emit_ln(ci, emit_xdma(ci))

            def s_ua1():
                for m in range(0, 8):
                    emit_uprojA(ci, m, st["xnT"])

            def s_ua2():
                for m in range(8, 16):
                    emit_uprojA(ci, m, st["xnT"])

            def s_cv1():
                for m in range(0, 8):
                    emit_conv(ci, m, S)

            def s_cv2():
                for m in range(8, 16):
                    emit_conv(ci, m, S)

            def s_xp_dt():
                for m in range(NMU):
                    emit_xproj(ci, m, pxp, S)
                emit_dt(ci, pxp, S)

            def s_bc():
                st["bc"] = emit_bcast(ci, S)

            def s_z():
                emit_z(ci, st["xnT"], S)

            return [s_ln, s_ua1, s_ua2, s_cv1, s_cv2, s_xp_dt, s_bc, s_z], st

        # ---------------- emission ----------------
        Ss = [{"u_silu": [None] * NMH, "dt_sb": [None] * NMH,
               "z_silu": [None] * NMH, "ygate": [None] * NMH}
              for _ in range(NCH)]
        # FRONT(0): fully exposed; PE-dense ordering, z after dt/bcast so the
        # first scan starts as early as possible.
        xts0 = emit_xdma(0)
        emit_consts()
        xnT0 = emit_ln(0, xts0)
        pxp0 = ppool.tile((96, LCMAX), FP32, name="pxp", tag="pxp", bufs=1)
        for m in range(NMU):
            emit_uprojA(0, m, xnT0)
        for m in range(NMU):
            emit_conv(0, m, Ss[0])
        for m in range(NMU):
            emit_xproj(0, m, pxp0, Ss[0])
        emit_dt(0, pxp0, Ss[0])
        bcs = {0: emit_bcast(0, Ss[0])}
        emit_z(0, xnT0, Ss[0])

        for ci in range(NCH):
            S = Ss[ci]
            nxt, nxt_st = (front_slices(ci + 1, Ss[ci + 1])
                           if ci + 1 < NCH else (None, None))
            pend = None  # (m, yps) with its gate not yet emitted
            for m in range(NMH):
                yps = emit_slab(ci, m, bcs[ci], S)
                if ci == 0 and m == 0:
                    emit_z(0, xnT0, Ss[0])
                if pend is not None:
                    emit_gate(ci, pend[0], S, pend[1])
                pend = (m, yps)
                # spread the next front one slice per slab so its scalar
                # ops do not starve the asl exps feeding the scans.
                if nxt is not None:
                    nxt[m]()
                # spread the previous chunk's out-proj two kk per slab so
                # its PE block does not delay this chunk's yps reduces.
                if ci > 0 and m < 4:
                    emit_outproj_kk(ci - 1, 2 * m, Ss[ci - 1])
                    emit_outproj_kk(ci - 1, 2 * m + 1, Ss[ci - 1])
            emit_gate(ci, pend[0], S, pend[1])
            if nxt is not None:
                bcs[ci + 1] = nxt_st["bc"]
        for kk in range(NKM):
            emit_outproj_kk(NCH - 1, kk, Ss[NCH - 1])

        dramp.release()
        scanp.release()
        front.release()
        trpool.release()
        ypool.release()
        ppool.release()
        wpool.release()
        main.release()
        const.release()
    if finalize:
        nc.finalize()
    return nc


def _shards(inputs):
    """Build the 8 per-core input maps (numpy, fp32/bf16 via ml_dtypes)."""
    import ml_dtypes

    def bf(a):
        return np.asarray(a, np.float32).astype(ml_dtypes.bfloat16)

    x = np.asarray(inputs["x"], np.float32)
    g = np.asarray(inputs["ln_g"], np.float32)
    be = np.asarray(inputs["ln_b"], np.float32)
    ident = np.eye(P, dtype=np.float32)

    maps = []
    for d, pre in ((0, "f_"), (1, "b_")):
        in_w = np.asarray(inputs[pre + "in_w"], np.float32)
        conv_w = np.asarray(inputs[pre + "conv_w"], np.float32)
        conv_b = np.asarray(inputs[pre + "conv_b"], np.float32)
        xproj_w = np.asarray(inputs[pre + "xproj_w"], np.float32)
        dt_w = np.asarray(inputs[pre + "dt_w"], np.float32)
        dt_b = np.asarray(inputs[pre + "dt_b"], np.float32)
        Alog = np.asarray(inputs[pre + "Alog"], np.float32)
        Dv = np.asarray(inputs[pre + "D"], np.float32)
        out_w = np.asarray(inputs[pre + "out_w"], np.float32)
        A = -np.exp(Alog)  # (DI, DSTATE)

        for b in range(2):
            for h in range(2):
                sl = slice(h * DH, (h + 1) * DH)
                # u channel tiles reordered so this core's half comes first
                order = np.r_[h * DH:(h + 1) * DH, (1 - h) * DH:(2 - h) * DH] if h == 1 else np.arange(DI)
                w_u = in_w[:DI][order] * g[None, :]
                w_z = in_w[DI:][sl] * g[None, :]
                bu_full = (in_w[:DI][order] @ be)
                bz_full = (in_w[DI:][sl] @ be)
                cw = conv_w[order]
                cb = conv_b[order]
                convdiag = np.zeros((NMU, P, DCONV, P), np.float32)
                for k in range(DCONV):
                    for m in range(NMU):
                        np.fill_diagonal(convdiag[m, :, k, :], cw[m * P:(m + 1) * P, k])
                convdiag = convdiag.reshape(NMU, P, DCONV * P)
                Ah = A[sl]  # (DH, 16)
                Amat_ = Ah.reshape(NMH, P, DSTATE).transpose(1, 0, 2).reshape(P, P)
                Dh = Dv[sl]
                Ddiag_ = np.zeros((NMH, P, P), np.float32)
                for m in range(NMH):
                    np.fill_diagonal(Ddiag_[m], Dh[m * P:(m + 1) * P])
                xp = xproj_w[:, order]  # (96, DI)
                xs = x[b] if d == 0 else x[b][::-1]
                m = {
                    "xin": np.ascontiguousarray(xs),
                    "wuT": bf(np.ascontiguousarray(
                        w_u.T.reshape(NKM, P, NMU, P).transpose(2, 1, 0, 3)
                        .reshape(NMU, P, NKM * P))),
                    "wzT": bf(np.ascontiguousarray(
                        w_z.T.reshape(NKM, P, NMH, P).transpose(2, 1, 0, 3)
                        .reshape(NMH, P, NKM * P))),
                    "bu": np.ascontiguousarray(bu_full.reshape(NMU, P).T),
                    "bz": np.ascontiguousarray(bz_full.reshape(NMH, P).T),
                    "convd": bf(convdiag),
                    "convb": np.ascontiguousarray(cb.reshape(NMU, P).T),
                    "wxpT": bf(np.ascontiguousarray(
                        xp.T.reshape(NMU, P, 96).transpose(1, 0, 2)
                        .reshape(P, NMU * 96))),
                    "wdtT": bf(dt_w[sl].T),
                    "dtb": np.ascontiguousarray(dt_b[sl].reshape(NMH, P).T),
                    "Amat": np.ascontiguousarray(Amat_),
                    "Ddiag": bf(Ddiag_),
                    "woT": bf(np.ascontiguousarray(
                        out_w[:, sl].T.reshape(NMH, P, NKM, P).transpose(2, 1, 0, 3)
                        .reshape(NKM, P, NMH * P))),
                    "ident": bf(ident),
                }
                maps.append(m)
    return maps


_CACHE = {}


def kernel(**inputs):
    if "nc" not in _CACHE:
        _CACHE["nc"] = build_program()
    nc = _CACHE["nc"]
    maps = _shards(inputs)
    res = run_bass_kernel_spmd(nc, maps, list(range(8)))
    _CACHE["last_res"] = res
    outs = res.results
    x = np.asarray(inputs["x"], np.float32)
    out = x.copy()
    i = 0
    for d in range(2):
        for b in range(2):
            for h in range(2):
                part = outs[i]["outp"].T  # (t, dmo)
                if d == 1:
                    part = part[::-1]
                out[b] += part
                i += 1
    return out


# revision 12
# speedup vs baseline: 1.1626x; 1.1626x over previous
"""BiMamba block Trainium2 kernel (v3: 3-chunk pipeline, DMA bcast,
gpsimd ch-multiply).

Sharding: 8 cores = (direction f/b) x (batch 0/1) x (d_inner half 0/1),
fully independent (no collectives).  Host flips the sequence for the
backward cores, relabels u channel tiles so this core's half occupies
m=0..7, and sums the 8 partial outputs + residual.

Per core the sequence is processed in 3 time chunks [256, 384, 384] so
the exposed PE-heavy front of chunk 0 is small and the front of chunk
c+1 overlaps the DVE-bound scan of chunk c.  The front emission for
chunk c+1 is sliced across the slab loop of chunk c so its scalar ops
do not head-of-line block the asl exps feeding the scans.

The selective scan runs per (chunk, state-group g, m-tile): d on
partitions, 8 state segments x LC t in the free dim, one
tensor_tensor_scan per slab; chunk boundaries are stitched by folding
a*h_prev into the first column of b.  B/C rows are broadcast to 128
partitions via a DRAM bounce + replicated-read DMAs (no PE/scalar).
The C*h multiply runs on gpsimd; the n-segment reduction
sum_j C_j*h_j + D*u runs on the PE as accumulating identity/diag
matmuls into PSUM.
"""

import sys

sys.path.insert(0, "/opt/trn_rl_repo")

import numpy as np

import concourse.bass as bass
import concourse.mybir as mybir
from concourse import bacc
from concourse.tile import TileContext
from concourse.bass_utils import run_bass_kernel_spmd

FP32 = mybir.dt.float32
BF16 = mybir.dt.bfloat16
AX = mybir.AxisListType
OP = mybir.AluOpType
AF = mybir.ActivationFunctionType

P = 128
L = 1024          # sequence length
DM = 1024         # d_model
DI = 2048         # d_inner
DH = 1024         # d_inner half per core
DSTATE = 16
DTRANK = 64
DCONV = 4
NKM = DM // P     # 8 d_model tiles
NMU = DI // P     # 16 u M-tiles
NMH = DH // P     # 8 half M-tiles
GSEG = 8          # states per scan slab
CHUNKS = [(0, 256), (256, 384), (640, 384)]
NCH = len(CHUNKS)
LCMAX = max(lc for _, lc in CHUNKS)
NTCMAX = (LCMAX + P - 1) // P
SLABMAX = GSEG * LCMAX


def build_program(finalize=True):
    nc = bacc.Bacc("TRN2", target_bir_lowering=False, debug=False)

    # ---- DRAM I/O (per-core shards; same names on every core) ----
    xin = nc.dram_tensor("xin", (L, DM), FP32, kind="ExternalInput")
    wuT = nc.dram_tensor("wuT", (NMU, P, NKM * P), BF16, kind="ExternalInput")
    wzT = nc.dram_tensor("wzT", (NMH, P, NKM * P), BF16, kind="ExternalInput")
    bu = nc.dram_tensor("bu", (P, NMU), FP32, kind="ExternalInput")
    bz = nc.dram_tensor("bz", (P, NMH), FP32, kind="ExternalInput")
    convd = nc.dram_tensor("convd", (NMU, P, DCONV * P), BF16, kind="ExternalInput")
    convb = nc.dram_tensor("convb", (P, NMU), FP32, kind="ExternalInput")
    wxpT = nc.dram_tensor("wxpT", (P, NMU * 96), BF16, kind="ExternalInput")
    wdtT = nc.dram_tensor("wdtT", (DTRANK, DH), BF16, kind="ExternalInput")
    dtb = nc.dram_tensor("dtb", (P, NMH), FP32, kind="ExternalInput")
    Amat = nc.dram_tensor("Amat", (P, P), FP32, kind="ExternalInput")
    Ddiag = nc.dram_tensor("Ddiag", (NMH, P, P), BF16, kind="ExternalInput")
    woT = nc.dram_tensor("woT", (NKM, P, NMH * P), BF16, kind="ExternalInput")
    ident = nc.dram_tensor("ident", (P, P), BF16, kind="ExternalInput")
    outp = nc.dram_tensor("outp", (DM, L), FP32, kind="ExternalOutput")

    with TileContext(nc) as tc:
        const = tc.alloc_tile_pool(name="const", bufs=1)
        main = tc.alloc_tile_pool(name="main", bufs=1)
        wpool = tc.alloc_tile_pool(name="wpool", bufs=2)
        ppool = tc.alloc_tile_pool(name="ppool", bufs=3, space="PSUM")
        ypool = tc.alloc_tile_pool(name="ypool", bufs=2, space="PSUM")
        trpool = tc.alloc_tile_pool(name="trpool", bufs=2, space="PSUM")
        front = tc.alloc_tile_pool(name="front", bufs=2)
        scanp = tc.alloc_tile_pool(name="scanp", bufs=1)
        dramp = tc.alloc_tile_pool(name="dramp", bufs=2, space="DRAM")

        bu_t = const.tile((P, NMU), FP32, name="bu_t")
        bz_t = const.tile((P, NMH), FP32, name="bz_t")
        convb_t = const.tile((P, NMU), FP32, name="convb_t")
        dtb_t = const.tile((P, NMH), FP32, name="dtb_t")
        A_t = const.tile((P, P), FP32, name="A_t")
        id_t = const.tile((P, P), BF16, name="id_t")
        dd_t = const.tile((P, NMH * P), BF16, name="dd_t")
        wdt = const.tile((DTRANK, DH), BF16, name="wdt")
        wxp_t = const.tile((P, NMU * 96), BF16, name="wxp_t")

        def emit_consts():
            for dst, srct in ((bu_t, bu), (bz_t, bz), (convb_t, convb),
                              (dtb_t, dtb), (A_t, Amat), (id_t, ident)):
                nc.sync.dma_start(out=dst[:], in_=srct[:])
            for m in range(NMH):
                nc.sync.dma_start(out=dd_t[:, m * P:(m + 1) * P], in_=Ddiag[m])
            nc.sync.dma_start(out=wdt[:], in_=wdtT[:])
            nc.sync.dma_start(out=wxp_t[:], in_=wxpT[:])

        # persistent across chunks
        u_pre = [main.tile((P, LCMAX + 4), BF16, name=f"up{m}", tag=f"up{m}")
                 for m in range(NMU)]
        hlast = [main.tile((P, DSTATE), BF16, name=f"hl{m}", tag=f"hl{m}")
                 for m in range(NMH)]

        # ---------------- front phase builders ----------------
        def emit_xdma(ci):
            c0, LC = CHUNKS[ci]
            xts = []
            for tt in range(LC // P):
                t0 = c0 + tt * P
                xt = front.tile((P, DM), FP32, name="xt", tag="xt", bufs=3)
                for kk in range(NKM):
                    nc.sync.dma_start(out=xt[:, kk * P:(kk + 1) * P],
                                      in_=xin[t0:t0 + P, kk * P:(kk + 1) * P])
                xts.append(xt)
            return xts

        def emit_ln(ci, xts):
            c0, LC = CHUNKS[ci]
            ntc = LC // P
            xnT = [front.tile((P, LCMAX), BF16, name=f"xnT{k}", tag=f"xnT{k}",
                              bufs=2) for k in range(NKM)]
            mus = front.tile((P, NTCMAX), FP32, name="mus", tag="mus", bufs=2)
            vars_ = front.tile((P, NTCMAX), FP32, name="vars", tag="vars",
                               bufs=2)
            for tt in range(ntc):
                xt = xts[tt]
                bns = front.tile((P, 12), FP32, name="bns", tag="stats", bufs=8)
                nc.vector.bn_stats(bns[:, 0:6], xt[:, 0:DM // 2])
                nc.vector.bn_stats(bns[:, 6:12], xt[:, DM // 2:DM])
                mv = front.tile((P, 2), FP32, name="mv", tag="stats", bufs=8)
                nc.vector.bn_aggr(mv[:], bns[:])
                nc.vector.tensor_copy(mus[:, tt:tt + 1], mv[:, 0:1])
                nc.vector.tensor_scalar_add(vars_[:, tt:tt + 1], mv[:, 1:2],
                                            1e-5)
            sds = front.tile((P, NTCMAX), FP32, name="sds", tag="stats", bufs=8)
            nc.scalar.activation(sds[:, :ntc], vars_[:, :ntc], AF.Sqrt)
            rs = front.tile((P, NTCMAX), FP32, name="rs", tag="rs", bufs=2)
            nc.vector.reciprocal(rs[:, :ntc], sds[:, :ntc])
            for tt in range(ntc):
                xn = front.tile((P, DM), BF16, name="xn", tag="xn", bufs=2)
                nc.vector.tensor_scalar(xn[:], xts[tt][:], mus[:, tt:tt + 1],
                                        rs[:, tt:tt + 1], OP.subtract, OP.mult)
                for kk in range(NKM):
                    tr = trpool.tile((P, P), BF16, name="tr", tag="tr")
                    nc.tensor.transpose(tr[:], xn[:, kk * P:(kk + 1) * P],
                                        id_t[:])
                    if ci == 0:
                        nc.vector.tensor_copy(
                            xnT[kk][:, tt * P:(tt + 1) * P], tr[:])
                    else:
                        nc.scalar.copy(xnT[kk][:, tt * P:(tt + 1) * P], tr[:])
            return xnT

        def emit_uprojA(ci, m, xnT):
            """in_proj u tile m -> u_pre (pre-conv)."""
            c0, LC = CHUNKS[ci]
            wt = wpool.tile((P, NKM * P), BF16, name="wt", tag="wt", bufs=2)
            nc.sync.dma_start(out=wt[:], in_=wuT[m])
            ps = ppool.tile((P, LCMAX), FP32, name="ps", tag="mm")
            for k in range(NKM):
                nc.tensor.matmul(ps[:, :LC], wt[:, k * P:(k + 1) * P],
                                 xnT[k][:, :LC],
                                 start=(k == 0), stop=(k == NKM - 1))
            if ci == 0:
                nc.vector.memset(u_pre[m][:, 0:4], 0.0)
            else:
                lcp = CHUNKS[ci - 1][1]
                nc.scalar.copy(u_pre[m][:, 1:4], u_pre[m][:, lcp + 1:lcp + 4])
            nc.scalar.activation(u_pre[m][:, 4:LC + 4], ps[:, :LC],
                                 AF.Identity, bias=bu_t[:, m:m + 1])

        def emit_conv(ci, m, S):
            """conv + silu of u tile m."""
            c0, LC = CHUNKS[ci]
            pc = ppool.tile((P, LCMAX), FP32, name="pc", tag="mm")
            cw = wpool.tile((P, DCONV * P), BF16, name="cw", tag="cw", bufs=2)
            nc.sync.dma_start(out=cw[:], in_=convd[m])
            for k in range(DCONV):
                nc.tensor.matmul(pc[:, :LC], cw[:, k * P:(k + 1) * P],
                                 u_pre[m][:, k + 1:k + 1 + LC],
                                 start=(k == 0), stop=(k == DCONV - 1))
            if m < NMH:
                us = front.tile((P, LCMAX), BF16, name=f"usl{m}",
                                tag=f"usl{m}", bufs=2)
                S["u_silu"][m] = us
            else:
                us = front.tile((P, LCMAX), BF16, name="uslB", tag="uslB",
                                bufs=2)
            S.setdefault("u_all", [None] * NMU)[m] = us
            nc.scalar.activation(us[:, :LC], pc[:, :LC], AF.Silu,
                                 bias=convb_t[:, m:m + 1])

        def emit_xproj(ci, m, pxp, S):
            c0, LC = CHUNKS[ci]
            nc.tensor.matmul(pxp[:, :LC], wxp_t[:, m * 96:(m + 1) * 96],
                             S["u_all"][m][:, :LC],
                             start=(m == 0), stop=(m == NMU - 1))

        def emit_dt(ci, pxp, S):
            c0, LC = CHUNKS[ci]
            dbc = front.tile((96, LCMAX), BF16, name="dbc", tag="dbc", bufs=2)
            S["dbc"] = dbc
            nc.scalar.copy(dbc[:, :LC], pxp[:, :LC])
            for m in range(NMH):
                psd = ppool.tile((P, LCMAX), FP32, name="psd", tag="mm")
                nc.tensor.matmul(psd[:, :LC], wdt[:, m * P:(m + 1) * P],
                                 dbc[0:DTRANK, :LC], start=True, stop=True)
                dts = front.tile((P, LCMAX), BF16, name=f"dts{m}",
                                 tag=f"dts{m}", bufs=2)
                S["dt_sb"][m] = dts
                nc.scalar.activation(dts[:, :LC], psd[:, :LC], AF.Exp,
                                     bias=dtb_t[:, m:m + 1])
            for m in range(NMH):
                dts = S["dt_sb"][m]
                nc.scalar.activation(dts[:, :LC], dts[:, :LC], AF.Ln, bias=1.0)

        def emit_z(ci, xnT, S):
            c0, LC = CHUNKS[ci]
            for m in range(NMH):
                wt = wpool.tile((P, NKM * P), BF16, name="wtz", tag="wt",
                                bufs=2)
                nc.sync.dma_start(out=wt[:], in_=wzT[m])
                ps = ppool.tile((P, LCMAX), FP32, name="psz", tag="mm")
                for k in range(NKM):
                    nc.tensor.matmul(ps[:, :LC], wt[:, k * P:(k + 1) * P],
                                     xnT[k][:, :LC],
                                     start=(k == 0), stop=(k == NKM - 1))
                zs = front.tile((P, LCMAX), BF16, name=f"zsl{m}",
                                tag=f"zsl{m}", bufs=2)
                S["z_silu"][m] = zs
                nc.scalar.activation(zs[:, :LC], ps[:, :LC], AF.Silu,
                                     bias=bz_t[:, m:m + 1])

        # ---------------- scan phase builders ----------------
        def emit_bcast(ci, S):
            """Broadcast the 32 B/C rows of dbc to 128 partitions via a DRAM
            bounce + replicated-read DMAs (no PE / scalar involvement)."""
            c0, LC = CHUNKS[ci]
            dbc = S["dbc"]
            bcd = dramp.tile((32, LCMAX), BF16, name="bcd", tag="bcd", bufs=2)
            nc.sync.dma_start(out=bcd[:, :LC], in_=dbc[DTRANK:DTRANK + 32, :LC])
            slabs = []
            for g in range(2):
                Bsl = scanp.tile((P, SLABMAX), BF16, name="Bsl",
                                 tag=f"B{g}{ci % 2}", bufs=1)
                Csl = scanp.tile((P, SLABMAX), BF16, name="Csl",
                                 tag=f"C{g}{ci % 2}", bufs=1)
                for j in range(GSEG):
                    n = g * GSEG + j
                    nc.sync.dma_start(
                        out=Bsl[:, j * LC:(j + 1) * LC],
                        in_=bcd[n:n + 1, :LC].partition_broadcast(P))
                    nc.sync.dma_start(
                        out=Csl[:, j * LC:(j + 1) * LC],
                        in_=bcd[16 + n:16 + n + 1, :LC].partition_broadcast(P))
                slabs.append((Bsl, Csl))
            return slabs

        def emit_slab(ci, m, bc, S):
            """Both state groups of m-tile m for chunk ci + PE reduce + gate."""
            c0, LC = CHUNKS[ci]
            SLABF = GSEG * LC
            dtu = scanp.tile((P, LCMAX), BF16, name="dtu", tag="dtu", bufs=2)
            nc.vector.tensor_mul(dtu[:, :LC], S["dt_sb"][m][:, :LC],
                                 S["u_silu"][m][:, :LC])
            yps = ypool.tile((P, LCMAX), FP32, name="yps", tag="yps")
            for g in range(2):
                Bsl, Csl = bc[g]
                asl = scanp.tile((P, SLABMAX), BF16, name="asl", tag="asl",
                                 bufs=2)
                for j in range(GSEG):
                    n = g * GSEG + j
                    nc.scalar.activation(
                        asl[:, j * LC:(j + 1) * LC], S["dt_sb"][m][:, :LC],
                        AF.Exp,
                        scale=A_t[:, m * DSTATE + n:m * DSTATE + n + 1])
                bsl = scanp.tile((P, SLABMAX), BF16, name="bsl", tag="bsl",
                                 bufs=2)
                nc.vector.tensor_tensor(
                    bsl[:, :SLABF].rearrange("p (j t) -> p j t", j=GSEG),
                    dtu[:, :LC].unsqueeze(1).to_broadcast((P, GSEG, LC)),
                    Bsl[:, :SLABF].rearrange("p (j t) -> p j t", j=GSEG),
                    OP.mult)
                if ci == 0:
                    nc.vector.memset(asl[:, 0:SLABF:LC], 0.0)
                else:
                    tmp8 = scanp.tile((P, GSEG), BF16, name="tmp8", tag="tmp8",
                                      bufs=2)
                    nc.vector.tensor_tensor(tmp8[:], asl[:, 0:SLABF:LC],
                                            hlast[m][:, g * GSEG:(g + 1) * GSEG],
                                            OP.mult)
                    nc.vector.tensor_tensor(bsl[:, 0:SLABF:LC],
                                            bsl[:, 0:SLABF:LC], tmp8[:],
                                            OP.add)
                    nc.vector.memset(asl[:, 0:SLABF:LC], 0.0)
                hsl = scanp.tile((P, SLABMAX), BF16, name="hsl", tag="hsl",
                                 bufs=2)
                nc.vector.tensor_tensor_scan(hsl[:, :SLABF], asl[:, :SLABF],
                                             bsl[:, :SLABF], 0.0,
                                             OP.mult, OP.add)
                if ci < NCH - 1:
                    nc.vector.tensor_copy(
                        hlast[m][:, g * GSEG:(g + 1) * GSEG],
                        hsl[:, LC - 1:SLABF:LC])
                ch = scanp.tile((P, SLABMAX), BF16, name="ch", tag="chs",
                                bufs=2)
                nc.vector.tensor_mul(ch[:, :SLABF], hsl[:, :SLABF],
                                     Csl[:, :SLABF])
                for j in range(GSEG):
                    nc.tensor.matmul(yps[:, :LC], id_t[:],
                                     ch[:, j * LC:(j + 1) * LC],
                                     start=(g == 0 and j == 0), stop=False)
            nc.tensor.matmul(yps[:, :LC], dd_t[:, m * P:(m + 1) * P],
                             S["u_silu"][m][:, :LC], start=False, stop=True)
            return yps

        def emit_gate(ci, m, S, yps):
            c0, LC = CHUNKS[ci]
            yg = front.tile((P, LCMAX), BF16, name=f"yg{m}", tag=f"yg{m}",
                            bufs=1)
            S["ygate"][m] = yg
            nc.vector.tensor_tensor(yg[:, :LC], yps[:, :LC],
                                    S["z_silu"][m][:, :LC], OP.mult)

        def emit_outproj_kk(ci, kk, S):
            c0, LC = CHUNKS[ci]
            wt = wpool.tile((P, NMH * P), BF16, name="wto", tag="wto",
                            bufs=2)
            nc.sync.dma_start(out=wt[:], in_=woT[kk])
            po = ppool.tile((P, LCMAX), FP32, name="po", tag="mm")
            for m in range(NMH):
                nc.tensor.matmul(po[:, :LC], wt[:, m * P:(m + 1) * P],
                                 S["ygate"][m][:, :LC],
                                 start=(m == 0), stop=(m == NMH - 1))
            osb = front.tile((P, LCMAX), FP32, name="osb", tag="osb",
                             bufs=2)
            nc.scalar.copy(osb[:, :LC], po[:, :LC])
            nc.sync.dma_start(out=outp[kk * P:(kk + 1) * P, c0:c0 + LC],
                              in_=osb[:, :LC])

        def front_slices(ci, S):
            """Emission slices for chunk ci's front, to be spread across the
            previous chunk's slab loop (keeps scalar ahead of the DVE)."""
            pxp = ppool.tile((96, LCMAX), FP32, name="pxp", tag="pxp", bufs=1)
            st = {}

            def s_ln():
                st["xnT"] = emit_ln(ci, emit_xdma(ci))

            def s_ua1():
                for m in range(0, 8):
                    emit_uprojA(ci, m, st["xnT"])

            def s_ua2():
                for m in range(8, 16):
                    emit_uprojA(ci, m, st["xnT"])

            def s_cv1():
                for m in range(0, 8):
                    emit_conv(ci, m, S)

            def s_cv2():
                for m in range(8, 16):
                    emit_conv(ci, m, S)

            def s_xp_dt():
                for m in range(NMU):
                    emit_xproj(ci, m, pxp, S)
                emit_dt(ci, pxp, S)

            def s_bc():
                st["bc"] = emit_bcast(ci, S)

            def s_z():
                emit_z(ci, st["xnT"], S)

            return [s_ln, s_ua1, s_ua2, s_cv1, s_cv2, s_xp_dt, s_bc, s_z], st

        # ---------------- emission ----------------
        Ss = [{"u_silu": [None] * NMH, "dt_sb": [None] * NMH,
               "z_silu": [None] * NMH, "ygate": [None] * NMH}
              for _ in range(NCH)]
        # FRONT(0): fully exposed; PE-dense ordering, z after dt/bcast so the
        # first scan starts as early as possible.
        xts0 = emit_xdma(0)
        emit_consts()
        xnT0 = emit_ln(0, xts0)
        pxp0 = ppool.tile((96, LCMAX), FP32, name="pxp", tag="pxp", bufs=1)
        for m in range(NMU):
            emit_uprojA(0, m, xnT0)
        for m in range(NMU):
            emit_conv(0, m, Ss[0])
        for m in range(NMU):
            emit_xproj(0, m, pxp0, Ss[0])
        emit_dt(0, pxp0, Ss[0])
        bcs = {0: emit_bcast(0, Ss[0])}
        emit_z(0, xnT0, Ss[0])

        for ci in range(NCH):
            S = Ss[ci]
            nxt, nxt_st = (front_slices(ci + 1, Ss[ci + 1])
                           if ci + 1 < NCH else (None, None))
            pend = None  # (m, yps) with its gate not yet emitted
            for m in range(NMH):
                yps = emit_slab(ci, m, bcs[ci], S)
                if ci == 0 and m == 0:
                    emit_z(0, xnT0, Ss[0])
                if pend is not None:
                    emit_gate(ci, pend[0], S, pend[1])
                pend = (m, yps)
                # spread the next front one slice per slab so its scalar
                # ops do not starve the asl exps feeding the scans.
                if nxt is not None:
                    nxt[m]()
            emit_gate(ci, pend[0], S, pend[1])
            if nxt is not None:
                bcs[ci + 1] = nxt_st["bc"]
            for kk in range(NKM):
                emit_outproj_kk(ci, kk, S)

        dramp.release()
        scanp.release()
        front.release()
        trpool.release()
        ypool.release()
        ppool.release()
        wpool.release()
        main.release()
        const.release()
    if finalize:
        nc.finalize()
    return nc


def _shards(inputs):
    """Build the 8 per-core input maps (numpy, fp32/bf16 via ml_dtypes)."""
    import ml_dtypes

    def bf(a):
        return np.asarray(a, np.float32).astype(ml_dtypes.bfloat16)

    x = np.asarray(inputs["x"], np.float32)
    g = np.asarray(inputs["ln_g"], np.float32)
    be = np.asarray(inputs["ln_b"], np.float32)
    ident = np.eye(P, dtype=np.float32)

    maps = []
    for d, pre in ((0, "f_"), (1, "b_")):
        in_w = np.asarray(inputs[pre + "in_w"], np.float32)
        conv_w = np.asarray(inputs[pre + "conv_w"], np.float32)
        conv_b = np.asarray(inputs[pre + "conv_b"], np.float32)
        xproj_w = np.asarray(inputs[pre + "xproj_w"], np.float32)
        dt_w = np.asarray(inputs[pre + "dt_w"], np.float32)
        dt_b = np.asarray(inputs[pre + "dt_b"], np.float32)
        Alog = np.asarray(inputs[pre + "Alog"], np.float32)
        Dv = np.asarray(inputs[pre + "D"], np.float32)
        out_w = np.asarray(inputs[pre + "out_w"], np.float32)
        A = -np.exp(Alog)  # (DI, DSTATE)

        for b in range(2):
            for h in range(2):
                sl = slice(h * DH, (h + 1) * DH)
                # u channel tiles reordered so this core's half comes first
                order = np.r_[h * DH:(h + 1) * DH, (1 - h) * DH:(2 - h) * DH] if h == 1 else np.arange(DI)
                w_u = in_w[:DI][order] * g[None, :]
                w_z = in_w[DI:][sl] * g[None, :]
                bu_full = (in_w[:DI][order] @ be)
                bz_full = (in_w[DI:][sl] @ be)
                cw = conv_w[order]
                cb = conv_b[order]
                convdiag = np.zeros((NMU, P, DCONV, P), np.float32)
                for k in range(DCONV):
                    for m in range(NMU):
                        np.fill_diagonal(convdiag[m, :, k, :], cw[m * P:(m + 1) * P, k])
                convdiag = convdiag.reshape(NMU, P, DCONV * P)
                Ah = A[sl]  # (DH, 16)
                Amat_ = Ah.reshape(NMH, P, DSTATE).transpose(1, 0, 2).reshape(P, P)
                Dh = Dv[sl]
                Ddiag_ = np.zeros((NMH, P, P), np.float32)
                for m in range(NMH):
                    np.fill_diagonal(Ddiag_[m], Dh[m * P:(m + 1) * P])
                xp = xproj_w[:, order]  # (96, DI)
                xs = x[b] if d == 0 else x[b][::-1]
                m = {
                    "xin": np.ascontiguousarray(xs),
                    "wuT": bf(np.ascontiguousarray(
                        w_u.T.reshape(NKM, P, NMU, P).transpose(2, 1, 0, 3)
                        .reshape(NMU, P, NKM * P))),
                    "wzT": bf(np.ascontiguousarray(
                        w_z.T.reshape(NKM, P, NMH, P).transpose(2, 1, 0, 3)
                        .reshape(NMH, P, NKM * P))),
                    "bu": np.ascontiguousarray(bu_full.reshape(NMU, P).T),
                    "bz": np.ascontiguousarray(bz_full.reshape(NMH, P).T),
                    "convd": bf(convdiag),
                    "convb": np.ascontiguousarray(cb.reshape(NMU, P).T),
                    "wxpT": bf(np.ascontiguousarray(
                        xp.T.reshape(NMU, P, 96).transpose(1, 0, 2)
                        .reshape(P, NMU * 96))),
                    "wdtT": bf(dt_w[sl].T),
                    "dtb": np.ascontiguousarray(dt_b[sl].reshape(NMH, P).T),
                    "Amat": np.ascontiguousarray(Amat_),
                    "Ddiag": bf(Ddiag_),
                    "woT": bf(np.ascontiguousarray(
                        out_w[:, sl].T.reshape(NMH, P, NKM, P).transpose(2, 1, 0, 3)
                        .reshape(NKM, P, NMH * P))),
                    "ident": bf(ident),
                }
                maps.append(m)
    return maps


_CACHE = {}


def kernel(**inputs):
    if "nc" not in _CACHE:
        _CACHE["nc"] = build_program()
    nc = _CACHE["nc"]
    maps = _shards(inputs)
    res = run_bass_kernel_spmd(nc, maps, list(range(8)))
    _CACHE["last_res"] = res
    outs = res.results
    x = np.asarray(inputs["x"], np.float32)
    out = x.copy()
    i = 0
    for d in range(2):
        for b in range(2):
            for h in range(2):
                part = outs[i]["outp"].T  # (t, dmo)
                if d == 1:
                    part = part[::-1]
                out[b] += part
                i += 1
    return out
